# revision 1
# baseline (speedup 1.0000x reference)
"""2-layer GCN + edge-logit decoder on 8 Trainium2 NeuronCores, v2.

The baseline was ~95% Q7/SWDGE descriptor generation (every gather and
scatter element costs ~6.5ns of serialized GpSimd time).  v2 removes
SWDGE work wherever the access pattern is host-known:

  L1: messages are raw x rows, so the host materializes the duplicated
      per-edge stream x[src[e]] (bf16, dst-block-major, self-loops as
      explicit edges).  The device streams it with static DMAs and
      segment-sums it per 128-edge tile with PE matmuls against
      (iota == dstl) * dis_src one-hot matrices built on DVE.  Using
      sum_e dis_s*x[s] @ W1 == sum_e dis_s*(x[s]@W1), the W1 matmul is
      applied per dst block after aggregation.
  L2: messages zn1[s] = dis_s*relu(z1pre[s]) are runtime values, so the
      dst-sharded cores gather them (SWDGE, unavoidable) from an
      AllGathered bf16 table -- but aggregation is again one-hot PE
      matmuls into PSUM (no dma_scatter_add), halving SWDGE elements.
  pairs: baseline machinery (fp32 z2 table, strided 64-float gathers).

Numerics: bf16 streams/tables, fp32 accumulation -> rel err ~1e-3.
"""

import math
import sys

import numpy as np
import ml_dtypes

for _p in ("/opt/trn_rl_repo",):
    if _p not in sys.path:
        sys.path.append(_p)

import concourse.bacc as bacc
import concourse.bass as bass
import concourse.mybir as mybir
import concourse.tile as tile
from concourse import bass_utils
from concourse.masks import make_identity

F32 = mybir.dt.float32
BF16 = mybir.dt.bfloat16
I16 = mybir.dt.int16
AF = mybir.ActivationFunctionType
ALU = mybir.AluOpType
BFNP = ml_dtypes.bfloat16


def default_cfg():
    return dict(
        N=100000,
        PAIRS=1000000,
        FEAT=128,
        HID=64,
        OUT=16,
        C=8,
        GCAP=9984,  # max L2 slots per dma_gather instruction
        TILE_F=7936,  # pairs per final gather instruction (62*128)
        DMA_SCRATCH=16384,
    )


def derive(cfg):
    d = dict(cfg)
    C = d["C"]
    d["S"] = int(math.ceil(d["N"] / C / 128)) * 128  # 12544
    d["NP"] = d["S"] * C
    d["GL"] = d["S"] // 128  # dst blocks per core (98)
    d["M2"] = ((d["N"] - 1) >> 2) + 1
    assert d["M2"] <= 32768
    need = 3 * d["OUT"] + d["M2"] * d["HID"]
    d["NTAB2F"] = max(d["NP"] * d["OUT"], int(math.ceil(need / 2048)) * 2048)
    # bf16 zn1 table: view (c,p) extends to c*65536*64 + p*64 + 32768*128
    d["ZTAB"] = 65536 * 64 + 64 + 32768 * 128 + 128
    return d


# ---------------------------------------------------------------- host prep


def _wrap16(arr):
    """[.., 16 subgroups, L] int16 -> dma_gather index layout, replicated
    to 128 partitions (8 Q7 cores)."""
    nsub, L = arr.shape[-2], arr.shape[-1]
    lead = arr.shape[:-2]
    a = arr.reshape(lead + (nsub, L // 16, 16))
    a = np.moveaxis(a, -1, -3)
    a = a.reshape(lead + (16, nsub * (L // 16)))
    return np.tile(a, (1,) * len(lead) + (8, 1)).astype(np.int16)


def _tileT(vals):
    """[slots] -> [128, T] tile-transposed layout (slot i -> [i%128, i//128])."""
    T = vals.shape[0] // 128
    return np.ascontiguousarray(vals.reshape(T, 128).T)


def prep_host(inputs, cfg):
    d = cfg
    N, C, S, NP = d["N"], d["C"], d["S"], d["NP"]
    FEAT, HID, OUT = d["FEAT"], d["HID"], d["OUT"]
    GL = d["GL"]
    TILE_F = d["TILE_F"]

    x = np.asarray(inputs["x"], np.float32)
    ei = np.asarray(inputs["edge_index"], np.int64)
    pe = np.asarray(inputs["pos_edge_index"], np.int64)
    ne = np.asarray(inputs["neg_edge_index"], np.int64)
    W1 = np.asarray(inputs["W1"], np.float32)
    b1 = np.asarray(inputs["b1"], np.float32)
    W2 = np.asarray(inputs["W2"], np.float32)
    b2 = np.asarray(inputs["b2"], np.float32)

    src, dst = ei[0], ei[1]

    x_bf = np.zeros((NP, FEAT), BFNP)
    x_bf[:N] = x.astype(BFNP)

    deg = (np.bincount(dst, minlength=NP) + 1.0).astype(np.float32)
    dis_h = 1.0 / np.sqrt(deg)  # graph-structure scalar, like baseline's deg
    degp_l = np.stack(
        [
            np.ascontiguousarray(deg[c * S : (c + 1) * S].reshape(GL, 128).T)
            for c in range(C)
        ]
    )

    # add self-loops as explicit edges
    allsrc = np.concatenate([src, np.arange(N, dtype=np.int64)])
    alldst = np.concatenate([dst, np.arange(N, dtype=np.int64)])
    core_of = alldst // S
    dstl = alldst - core_of * S  # local dst in [0, S)
    blk = dstl >> 7  # dst block in [0, GL)
    dib = dstl & 127  # dst-in-block

    # per-core edge lists sorted by block (stable keeps src order)
    per_core = []
    cnts = np.zeros((C, GL), np.int64)
    for c in range(C):
        m = core_of == c
        sc, bc, dc = allsrc[m], blk[m], dib[m]
        o = np.argsort(bc, kind="stable")
        sc, bc, dc = sc[o], bc[o], dc[o]
        cnts[c] = np.bincount(bc, minlength=GL)
        per_core.append((sc, bc, dc))

    T1 = np.maximum(1, (cnts.max(axis=0) + 127) // 128)  # tiles per block
    T1off = np.concatenate([[0], np.cumsum(T1)])
    T1tot = int(T1off[-1])
    SL1 = T1tot * 128

    # L2 subgroup of each edge slot: (src>>16)*2 + (src&1)
    # per (block, sg) tile counts, core-max
    cnts2 = np.zeros((C, GL, 4), np.int64)
    for c in range(C):
        sc, bc, _ = per_core[c]
        sg = ((sc >> 16) * 2 + (sc & 1)).astype(np.int64)
        np.add.at(cnts2[c], (bc, sg), 1)
    T2 = np.maximum(1, (cnts2.max(axis=0) + 127) // 128)  # [GL, 4] tiles
    # slot layout: sg-major, block-minor
    T2sg = T2.sum(axis=0)  # tiles per sg
    sgbase = np.concatenate([[0], np.cumsum(T2sg)]) * 128
    # within sg, block offsets
    blkoff2 = np.zeros((4, GL), np.int64)
    for s in range(4):
        blkoff2[s] = sgbase[s] + np.concatenate([[0], np.cumsum(T2[:, s])])[:-1] * 128
    SL2 = int(sgbase[-1])

    in_maps = []
    for c in range(C):
        sc, bc, dc = per_core[c]
        # ---- L1 slot assignment (block-major, pad each block to T1[b]*128)
        slot_src = np.zeros(SL1, np.int64)
        slot_d = np.full(SL1, -1.0, np.float32)
        slot_s = np.zeros(SL1, np.float32)
        boff = np.concatenate([[0], np.cumsum(cnts[c])])[:-1]
        pos = (T1off[bc] * 128) + (np.arange(sc.shape[0]) - boff[bc])
        slot_src[pos] = sc
        slot_d[pos] = dc
        slot_s[pos] = dis_h[sc]

        xe1 = x_bf[slot_src]  # [SL1, 128]
        xe1 = np.ascontiguousarray(
            xe1.reshape(T1tot, 128, FEAT).transpose(1, 0, 2).reshape(128, T1tot * FEAT)
        )
        oh1 = np.zeros((SL1, 128), BFNP)
        v1 = slot_d >= 0
        oh1[np.nonzero(v1)[0], slot_d[v1].astype(np.int64)] = slot_s[v1].astype(BFNP)
        oh1 = np.ascontiguousarray(
            oh1.reshape(T1tot, 128, 128).transpose(1, 0, 2).reshape(128, T1tot * 128)
        )

        # ---- L2 slot assignment (sg-major, block-minor, pad per (b, sg))
        sg = ((sc >> 16) * 2 + (sc & 1)).astype(np.int64)
        slot2_m = np.zeros(SL2, np.int64)
        slot2_d = np.full(SL2, -1.0, np.float32)
        # rank within (b, sg)
        key = bc * 4 + sg
        o2 = np.argsort(key, kind="stable")
        ks = key[o2]
        rank = np.arange(ks.shape[0]) - np.searchsorted(ks, ks)
        pos2 = np.empty_like(rank)
        pos2[o2] = blkoff2[sg[o2], bc[o2]] + rank
        slot2_m[pos2] = (sc & 65535) >> 1
        slot2_d[pos2] = dc
        gidx2 = np.concatenate(
            [
                _wrap16(slot2_m[sgbase[s] : sgbase[s + 1]].astype(np.int16)[None, :])
                for s in range(4)
            ],
            axis=1,
        )
        T2tot_ = SL2 // 128
        oh2 = np.zeros((SL2, 128), BFNP)
        v2 = slot2_d >= 0
        oh2[np.nonzero(v2)[0], slot2_d[v2].astype(np.int64)] = BFNP(1.0)
        oh2 = np.ascontiguousarray(
            oh2.reshape(T2tot_, 128, 128).transpose(1, 0, 2).reshape(128, T2tot_ * 128)
        )

        in_maps.append(
            dict(
                xe1=xe1,
                oh1=oh1,
                gidx2=np.ascontiguousarray(gidx2),
                oh2=oh2,
                degp_l=degp_l[c],
                w1=np.ascontiguousarray(W1.astype(BFNP)),
                w2=np.ascontiguousarray(W2),
                b1r=np.ascontiguousarray(np.tile(b1[None, :], (128, 1))),
                b2r=np.ascontiguousarray(np.tile(b2[None, :], (128, 1))),
            )
        )

    # ---- final pairs (baseline machinery)
    pq = np.concatenate([pe, ne], axis=1)
    P = pq.shape[1]
    PC = P // C
    a = pq[0].reshape(C, PC)
    b = pq[1].reshape(C, PC)
    fkey = (a & 3) * 4 + (b & 3)
    forder = np.argsort(fkey, axis=1, kind="stable")
    fks = np.take_along_axis(fkey, forder, axis=1)
    a_s = np.take_along_axis(a, forder, axis=1)
    b_s = np.take_along_axis(b, forder, axis=1)
    fbounds = np.stack([np.searchsorted(fks[c], np.arange(17)) for c in range(C)])
    fcounts = fbounds[:, 1:] - fbounds[:, :-1]
    TILE_F = int(math.ceil(fcounts.max() / 128)) * 128  # one instr per sg
    n_ft = 1
    F_sub = n_ft * TILE_F

    fA = np.empty((C, 16, F_sub), np.int16)
    fB = np.empty((C, 16, F_sub), np.int16)
    TJ = TILE_F // 128
    i = np.arange(F_sub)
    t_i = i // TILE_F
    r = i % TILE_F
    lin_i = t_i * TILE_F + (r % 128) * TJ + (r // 128)
    out_pos = np.empty((C, 16 * F_sub), np.int64)
    out_src = np.empty((C, 16 * F_sub), np.int64)
    for c in range(C):
        for s in range(16):
            b0, b1_ = fbounds[c, s], fbounds[c, s + 1]
            cnt = b1_ - b0
            pad = np.arange(F_sub - cnt, dtype=np.int64) % 128
            fA[c, s, :cnt] = a_s[c, b0:b1_] >> 2
            fA[c, s, cnt:] = pad
            fB[c, s, :cnt] = b_s[c, b0:b1_] >> 2
            fB[c, s, cnt:] = pad
            base = s * F_sub
            out_pos[c, base : base + F_sub] = s * n_ft * TILE_F + lin_i
            osrc = np.full(F_sub, -1, np.int64)
            osrc[:cnt] = c * PC + forder[c, b0:b1_]
            out_src[c, base : base + F_sub] = osrc
    fidxA = _wrap16(fA)
    fidxB = _wrap16(fB)
    for c in range(C):
        in_maps[c]["fidxA"] = np.ascontiguousarray(fidxA[c])
        in_maps[c]["fidxB"] = np.ascontiguousarray(fidxB[c])

    meta = dict(
        T1=tuple(int(t) for t in T1),
        T2=tuple(tuple(int(t) for t in row) for row in T2),
        n_ft=n_ft,
        TILE_F=TILE_F,
        P=P,
        out_pos=out_pos,
        out_src=out_src,
    )
    return in_maps, meta


def assemble(out_maps, meta, cfg):
    P = meta["P"]
    logits = np.zeros(P, np.float32)
    for c in range(cfg["C"]):
        lraw = out_maps[c]["lraw"].reshape(-1)
        pos = meta["out_pos"][c]
        srcg = meta["out_src"][c]
        valid = srcg >= 0
        logits[srcg[valid]] = lraw[pos[valid]]
    return logits


# ---------------------------------------------------------------- device build


def build(cfg, meta, enable_asserts=False):
    d = cfg
    C = d["C"]
    FEAT, HID, OUT = d["FEAT"], d["HID"], d["OUT"]
    S, NP, GL = d["S"], d["NP"], d["GL"]
    TILE_F = meta["TILE_F"]
    T1 = meta["T1"]
    T2 = meta["T2"]
    n_ft = meta["n_ft"]
    F_sub = n_ft * TILE_F
    TJ_F = TILE_F // 128
    T1tot = sum(T1)
    T2sg = [sum(T2[b][s] for b in range(GL)) for s in range(4)]
    T2tot = sum(T2sg)
    GCAP = d["GCAP"]

    nc = bacc.Bacc(
        "TRN2",
        target_bir_lowering=False,
        debug=False,
        enable_asserts=enable_asserts,
        num_devices=C,
        dynamic_dma_scratch_size=d["DMA_SCRATCH"],
        num_swdge_queues=2,
    )

    # I/O
    xe1 = nc.dram_tensor("xe1", [128, T1tot * FEAT], BF16, kind="ExternalInput")
    oh1 = nc.dram_tensor("oh1", [128, T1tot * 128], BF16, kind="ExternalInput")
    gidx2 = nc.dram_tensor("gidx2", [128, T2tot * 8], I16, kind="ExternalInput")
    oh2 = nc.dram_tensor("oh2", [128, T2tot * 128], BF16, kind="ExternalInput")
    degp_l = nc.dram_tensor("degp_l", [128, GL], F32, kind="ExternalInput")
    w1 = nc.dram_tensor("w1", [FEAT, HID], BF16, kind="ExternalInput")
    w2 = nc.dram_tensor("w2", [HID, OUT], F32, kind="ExternalInput")
    b1r = nc.dram_tensor("b1r", [128, HID], F32, kind="ExternalInput")
    b2r = nc.dram_tensor("b2r", [128, OUT], F32, kind="ExternalInput")
    fidxA = nc.dram_tensor("fidxA", [128, F_sub], I16, kind="ExternalInput")
    fidxB = nc.dram_tensor("fidxB", [128, F_sub], I16, kind="ExternalInput")
    lraw = nc.dram_tensor("lraw", [16 * F_sub], F32, kind="ExternalOutput")

    # internal DRAM
    zn1_sh = nc.dram_tensor("zn1_sh", [S * HID], BF16)
    zn1_t = nc.dram_tensor("zn1_t", [d["ZTAB"]], BF16, addr_space="Shared")
    z2_sh = nc.dram_tensor("z2_sh", [S * OUT], F32)
    z2_t = nc.dram_tensor("z2_t", [d["NTAB2F"]], F32, addr_space="Shared")

    groups = [list(range(C))]

    def zn1_view(sub):
        c, p = sub >> 1, sub & 1
        base = c * 65536 * HID + p * HID
        return zn1_t.ap()[base : base + 32768 * 128].rearrange("(m e) -> m e", e=128)

    def tab2_view(t, par):
        return t.ap()[par * OUT : par * OUT + d["M2"] * HID].rearrange(
            "(m e) -> m e", e=HID
        )

    with tile.TileContext(nc) as tc:
        with (
            tc.tile_pool(name="persist", bufs=1) as pP,
            tc.tile_pool(name="idx", bufs=4) as pIdx,
        ):
            # ---- persistent small tensors
            w1_sb = pP.tile([FEAT, HID], BF16)
            nc.sync.dma_start(out=w1_sb[:], in_=w1[:, :])
            w2_sb = pP.tile([HID, OUT], F32)
            nc.sync.dma_start(out=w2_sb[:], in_=w2[:, :])
            b1_sb = pP.tile([128, HID], F32)
            nc.sync.dma_start(out=b1_sb[:], in_=b1r[:, :])
            b2_sb = pP.tile([128, OUT], F32)
            nc.sync.dma_start(out=b2_sb[:], in_=b2r[:, :])
            ident = pP.tile([128, 128], F32)
            make_identity(nc, ident[:])

            dl_raw = pP.tile([128, GL], F32)
            nc.sync.dma_start(out=dl_raw[:], in_=degp_l[:, :])
            dis_l = pP.tile([128, GL], F32)
            nc.vector.reciprocal(dis_l[:], dl_raw[:])
            nc.scalar.activation(dis_l[:], dis_l[:], AF.Sqrt)


            # mid-lived tensors: freed before the final phase to fit SBUF
            with tc.tile_pool(name="mid", bufs=1) as pM:
                zn1_local = pM.tile([128, GL * HID], BF16)
                z2_local = pM.tile([128, GL * OUT], F32)
                t1T_sb = pM.tile([HID, S], F32)
                nc.vector.memset(t1T_sb[:], 0.0)

                # ---- zero z2 table tail (strided pair views read past NP*OUT)
                ZCOLS = 4096
                with tc.tile_pool(name="zero", bufs=1) as pZ:
                    zsb = pZ.tile([128, ZCOLS], F32)
                    nc.vector.memset(zsb[:], 0.0)
                    flat = z2_t.ap()
                    off = NP * OUT
                    n_floats = d["NTAB2F"] - off
                    assert n_floats % 128 == 0
                    while n_floats > 0:
                        f = min(ZCOLS, n_floats // 128)
                        nc.sync.dma_start(
                            out=flat[off : off + 128 * f].rearrange("(p f) -> p f", f=f),
                            in_=zsb[:, 0:f],
                        )
                        off += 128 * f
                        n_floats -= 128 * f

                # ---- L1: stream x_edge, one-hot aggregate, per-block epilogue
                with (
                    tc.tile_pool(name="l1s", bufs=3) as pS,
                    tc.tile_pool(name="l1oh", bufs=3) as pOh,
                    tc.tile_pool(name="l1e", bufs=3) as pC1,
                    tc.tile_pool(name="psA", bufs=2, space="PSUM") as psA,
                    tc.tile_pool(name="psE", bufs=2, space="PSUM") as psE,
                ):
                    coff = 0
                    for b in range(GL):
                        Tb = T1[b]
                        xe_sb = pS.tile([128, Tb * FEAT], BF16, tag="xe")
                        nc.sync.dma_start(
                            out=xe_sb[:], in_=xe1[:, coff * FEAT : (coff + Tb) * FEAT]
                        )
                        oh = pOh.tile([128, Tb * 128], BF16, tag="oh")
                        nc.sync.dma_start(
                            out=oh[:], in_=oh1[:, coff * 128 : (coff + Tb) * 128]
                        )
                        ps = psA.tile([128, 128], F32, tag="agg")
                        for t in range(Tb):
                            nc.tensor.matmul(
                                ps[:],
                                lhsT=xe_sb[:, t * FEAT : (t + 1) * FEAT],
                                rhs=oh[:, t * 128 : (t + 1) * 128],
                                start=(t == 0),
                                stop=(t == Tb - 1),
                            )
                        # epilogue: aggT [f, d] -> z1preT = W1^T@aggT -> transpose
                        aggT_sb = pC1.tile([128, 128], BF16, tag="aggT")
                        nc.vector.tensor_copy(aggT_sb[:], ps[:])
                        ps_z = psE.tile([HID, 128], F32, tag="psz")
                        nc.tensor.matmul(
                            ps_z[:], lhsT=w1_sb[:], rhs=aggT_sb[:], start=True, stop=True
                        )
                        zpT_sb = pC1.tile([HID, 128], F32, tag="zpT")
                        nc.vector.tensor_copy(zpT_sb[:], ps_z[:])
                        ps_t = psE.tile([128, HID], F32, tag="pst")
                        nc.tensor.transpose(ps_t[:], zpT_sb[:], ident[0:HID, 0:HID])
                        z1 = pC1.tile([128, HID], F32, tag="z1")
                        nc.vector.tensor_scalar(
                            out=z1[:],
                            in0=ps_t[:],
                            scalar1=dis_l[:, b : b + 1],
                            scalar2=None,
                            op0=ALU.mult,
                        )
                        nc.vector.tensor_tensor(out=z1[:], in0=z1[:], in1=b1_sb[:], op=ALU.add)
                        nc.scalar.activation(z1[:], z1[:], AF.Relu)
                        nc.vector.tensor_scalar(
                            out=zn1_local[:, b * HID : (b + 1) * HID],
                            in0=z1[:],
                            scalar1=dis_l[:, b : b + 1],
                            scalar2=None,
                            op0=ALU.mult,
                        )
                        coff += Tb

                nc.sync.dma_start(
                    out=zn1_sh.ap().rearrange("(g p f) -> p g f", p=128, f=HID),
                    in_=zn1_local[:].rearrange("p (g f) -> p g f", f=HID),
                )
                nc.gpsimd.collective_compute(
                    "AllGather",
                    ALU.bypass,
                    replica_groups=groups,
                    ins=[zn1_sh.ap()],
                    outs=[zn1_t.ap()[0 : NP * HID]],
                )

                # ---- L2: gather zn1 rows (block-grouped), one-hot aggregate
                with (
                    tc.tile_pool(name="msg", bufs=2) as pMsg,
                    tc.tile_pool(name="l2oh", bufs=2) as pOh2,
                    tc.tile_pool(name="psB", bufs=4, space="PSUM") as psB,
                ):
                    md2off = 0  # in tiles
                    qi = 0
                    for s in range(4):
                        # batch whole blocks into gather instructions <= GCAP slots
                        runs = []
                        run = []
                        slots = 0
                        for b in range(GL):
                            tb = T2[b][s]
                            if slots + tb * 128 > GCAP and run:
                                runs.append(run)
                                run, slots = [], 0
                            run.append(b)
                            slots += tb * 128
                        if run:
                            runs.append(run)
                        goff = sum(T2sg[ss] for ss in range(s)) * 8  # idx cols so far
                        for run in runs:
                            rslots = sum(T2[b][s] for b in run) * 128
                            gi = pIdx.tile([128, rslots // 16], I16, tag="gi")
                            nc.sync.dma_start(
                                out=gi[:], in_=gidx2[:, goff : goff + rslots // 16]
                            )
                            goff += rslots // 16
                            msg = pMsg.tile([128, rslots // 128, 128], BF16, tag="msg")
                            nc.gpsimd.dma_gather(
                                msg[:], zn1_view(s), gi[:], rslots, rslots, 128,
                                single_packet=rslots <= 1024,
                                queue_num=qi,
                            )
                            qi ^= 1
                            rtiles = rslots // 128
                            oh2_sb = pOh2.tile([128, rtiles * 128], BF16, tag="oh2")
                            nc.sync.dma_start(
                                out=oh2_sb[:],
                                in_=oh2[:, md2off * 128 : (md2off + rtiles) * 128],
                            )
                            j = 0
                            for b in run:
                                tb = T2[b][s]
                                ps2 = psB.tile([HID, 128], F32, tag="t1z")
                                for t in range(tb):
                                    nc.tensor.matmul(
                                        ps2[:],
                                        lhsT=msg[:, j + t, 0:HID],
                                        rhs=oh2_sb[:, (j + t) * 128 : (j + t + 1) * 128],
                                        start=(t == 0),
                                        stop=(t == tb - 1),
                                    )
                                nc.vector.tensor_tensor(
                                    out=t1T_sb[:, b * 128 : (b + 1) * 128],
                                    in0=t1T_sb[:, b * 128 : (b + 1) * 128],
                                    in1=ps2[:],
                                    op=ALU.add,
                                )
                                j += tb
                            md2off += j

                # ---- L2 epilogue per block: z2 = dis*(t1z @ W2) + b2
                with (
                    tc.tile_pool(name="l2e", bufs=3) as pC2,
                    tc.tile_pool(name="psF", bufs=2, space="PSUM") as psF,
                ):
                    for b in range(GL):
                        ps_q = psF.tile([OUT, 128], F32, tag="psq")
                        nc.tensor.matmul(
                            ps_q[:],
                            lhsT=w2_sb[:],
                            rhs=t1T_sb[:, b * 128 : (b + 1) * 128],
                            start=True,
                            stop=True,
                        )
                        q_sb = pC2.tile([OUT, 128], F32, tag="qsb")
                        nc.vector.tensor_copy(q_sb[:], ps_q[:])
                        ps_q2 = psF.tile([128, OUT], F32, tag="psq2")
                        nc.tensor.transpose(ps_q2[:], q_sb[:], ident[0:OUT, 0:OUT])
                        nc.vector.tensor_scalar(
                            out=z2_local[:, b * OUT : (b + 1) * OUT],
                            in0=ps_q2[:],
                            scalar1=dis_l[:, b : b + 1],
                            scalar2=None,
                            op0=ALU.mult,
                        )
                        nc.vector.tensor_tensor(
                            out=z2_local[:, b * OUT : (b + 1) * OUT],
                            in0=z2_local[:, b * OUT : (b + 1) * OUT],
                            in1=b2_sb[:],
                            op=ALU.add,
                        )

                nc.sync.dma_start(
                    out=z2_sh.ap().rearrange("(g p f) -> p g f", p=128, f=OUT),
                    in_=z2_local[:].rearrange("p (g f) -> p g f", f=OUT),
                )
                nc.gpsimd.collective_compute(
                    "AllGather",
                    ALU.bypass,
                    replica_groups=groups,
                    ins=[z2_sh.ap()],
                    outs=[z2_t.ap()[0 : NP * OUT]],
                )


            # ---- final: edge logits (baseline machinery)
            with tc.tile_pool(name="fin", bufs=3) as pFin:
                colsF = TILE_F // 16
                for s in range(16):
                    for t in range(n_ft):
                        off16 = (s * n_ft + t) * colsF
                        fa = pIdx.tile([128, colsF], I16, tag="fa")
                        nc.sync.dma_start(
                            out=fa[:], in_=fidxA[:, off16 : off16 + colsF]
                        )
                        fb = pIdx.tile([128, colsF], I16, tag="fb")
                        nc.sync.dma_start(
                            out=fb[:], in_=fidxB[:, off16 : off16 + colsF]
                        )
                        ma = pFin.tile([128, TJ_F, HID], F32, tag="ma")
                        nc.gpsimd.dma_gather(
                            ma[:], tab2_view(z2_t, s >> 2), fa[:], TILE_F, TILE_F, HID,
                            single_packet=TILE_F <= 1024,
                            queue_num=0,
                        )
                        mb = pFin.tile([128, TJ_F, HID], F32, tag="mb")
                        nc.gpsimd.dma_gather(
                            mb[:], tab2_view(z2_t, s & 3), fb[:], TILE_F, TILE_F, HID,
                            single_packet=TILE_F <= 1024,
                            queue_num=1,
                        )
                        prod = pFin.tile([128, TJ_F, OUT], F32, tag="prod")
                        nc.vector.tensor_tensor(
                            out=prod[:],
                            in0=ma[:, :, 0:OUT],
                            in1=mb[:, :, 0:OUT],
                            op=ALU.mult,
                        )
                        red = pFin.tile([128, TJ_F], F32, tag="red")
                        nc.vector.reduce_sum(
                            out=red[:, :, None],
                            in_=prod[:],
                            axis=mybir.AxisListType.X,
                        )
                        blk = s * n_ft + t
                        nc.sync.dma_start(
                            out=lraw.ap()[
                                blk * TILE_F : (blk + 1) * TILE_F
                            ].rearrange("(p j) -> p j", j=TJ_F),
                            in_=red[:],
                        )

    nc.compile()
    return nc


# ---------------------------------------------------------------- entry point

_CACHE = {}
TRACE = False
LAST = {}


def kernel(**inputs):
    cfg = derive(default_cfg())
    in_maps, meta = prep_host(inputs, cfg)
    key = (meta["T1"], meta["T2"], meta["n_ft"], meta["TILE_F"])
    if key not in _CACHE:
        _CACHE[key] = build(cfg, meta)
    nc = _CACHE[key]
    res = bass_utils.run_bass_kernel_spmd(
        nc, in_maps, core_ids=list(range(cfg["C"])), trace=TRACE
    )
    LAST["res"] = res
    return assemble(res.results, meta, cfg)



# revision 2
# speedup vs baseline: 1.2358x; 1.2358x over previous
"""2-layer GCN + edge-logit decoder on 8 Trainium2 NeuronCores, v2.

The baseline was ~95% Q7/SWDGE descriptor generation (every gather and
scatter element costs ~6.5ns of serialized GpSimd time).  v2 removes
SWDGE work wherever the access pattern is host-known:

  L1: messages are raw x rows, so the host materializes the duplicated
      per-edge stream x[src[e]] (bf16, dst-block-major, self-loops as
      explicit edges).  The device streams it with static DMAs and
      segment-sums it per 128-edge tile with PE matmuls against
      (iota == dstl) * dis_src one-hot matrices built on DVE.  Using
      sum_e dis_s*x[s] @ W1 == sum_e dis_s*(x[s]@W1), the W1 matmul is
      applied per dst block after aggregation.
  L2: messages zn1[s] = dis_s*relu(z1pre[s]) are runtime values, so the
      dst-sharded cores gather them (SWDGE, unavoidable) from an
      AllGathered bf16 table -- but aggregation is again one-hot PE
      matmuls into PSUM (no dma_scatter_add), halving SWDGE elements.
  pairs: baseline machinery (fp32 z2 table, strided 64-float gathers).

Numerics: bf16 streams/tables, fp32 accumulation -> rel err ~1e-3.
"""

import math
import sys

import numpy as np
import ml_dtypes

for _p in ("/opt/trn_rl_repo",):
    if _p not in sys.path:
        sys.path.append(_p)

import concourse.bacc as bacc
import concourse.bass as bass
import concourse.mybir as mybir
import concourse.tile as tile
from concourse import bass_utils
from concourse.masks import make_identity

F32 = mybir.dt.float32
BF16 = mybir.dt.bfloat16
I16 = mybir.dt.int16
AF = mybir.ActivationFunctionType
ALU = mybir.AluOpType
BFNP = ml_dtypes.bfloat16


def default_cfg():
    return dict(
        N=100000,
        PAIRS=1000000,
        FEAT=128,
        HID=64,
        OUT=16,
        C=8,
        GCAP=4992,  # max L2 slots per dma_gather instruction
        TILE_F=7936,  # pairs per final gather instruction (62*128)
        DMA_SCRATCH=16384,
    )


def derive(cfg):
    d = dict(cfg)
    C = d["C"]
    d["S"] = int(math.ceil(d["N"] / C / 128)) * 128  # 12544
    d["NP"] = d["S"] * C
    d["GL"] = d["S"] // 128  # dst blocks per core (98)
    d["M2"] = ((d["N"] - 1) >> 2) + 1
    assert d["M2"] <= 32768
    need = 3 * d["OUT"] + d["M2"] * d["HID"]
    d["NTAB2F"] = max(d["NP"] * d["OUT"], int(math.ceil(need / 2048)) * 2048)
    # bf16 zn1 table: view (c,p) extends to c*65536*64 + p*64 + 32768*128
    d["ZTAB"] = 65536 * 64 + 64 + 32768 * 128 + 128
    return d


# ---------------------------------------------------------------- host prep


def _wrap16(arr):
    """[.., 16 subgroups, L] int16 -> dma_gather index layout, replicated
    to 128 partitions (8 Q7 cores)."""
    nsub, L = arr.shape[-2], arr.shape[-1]
    lead = arr.shape[:-2]
    a = arr.reshape(lead + (nsub, L // 16, 16))
    a = np.moveaxis(a, -1, -3)
    a = a.reshape(lead + (16, nsub * (L // 16)))
    return np.tile(a, (1,) * len(lead) + (8, 1)).astype(np.int16)


def _tileT(vals):
    """[slots] -> [128, T] tile-transposed layout (slot i -> [i%128, i//128])."""
    T = vals.shape[0] // 128
    return np.ascontiguousarray(vals.reshape(T, 128).T)


def prep_host(inputs, cfg):
    d = cfg
    N, C, S, NP = d["N"], d["C"], d["S"], d["NP"]
    FEAT, HID, OUT = d["FEAT"], d["HID"], d["OUT"]
    GL = d["GL"]
    TILE_F = d["TILE_F"]

    x = np.asarray(inputs["x"], np.float32)
    ei = np.asarray(inputs["edge_index"], np.int64)
    pe = np.asarray(inputs["pos_edge_index"], np.int64)
    ne = np.asarray(inputs["neg_edge_index"], np.int64)
    W1 = np.asarray(inputs["W1"], np.float32)
    b1 = np.asarray(inputs["b1"], np.float32)
    W2 = np.asarray(inputs["W2"], np.float32)
    b2 = np.asarray(inputs["b2"], np.float32)

    src, dst = ei[0], ei[1]

    x_bf = np.zeros((NP, FEAT), BFNP)
    x_bf[:N] = x.astype(BFNP)

    deg = (np.bincount(dst, minlength=NP) + 1.0).astype(np.float32)
    dis_h = 1.0 / np.sqrt(deg)  # graph-structure scalar, like baseline's deg
    degp_l = np.stack(
        [
            np.ascontiguousarray(deg[c * S : (c + 1) * S].reshape(GL, 128).T)
            for c in range(C)
        ]
    )

    # add self-loops as explicit edges
    allsrc = np.concatenate([src, np.arange(N, dtype=np.int64)])
    alldst = np.concatenate([dst, np.arange(N, dtype=np.int64)])
    core_of = alldst // S
    dstl = alldst - core_of * S  # local dst in [0, S)
    blk = dstl >> 7  # dst block in [0, GL)
    dib = dstl & 127  # dst-in-block

    # per-core edge lists sorted by block (stable keeps src order)
    per_core = []
    cnts = np.zeros((C, GL), np.int64)
    for c in range(C):
        m = core_of == c
        sc, bc, dc = allsrc[m], blk[m], dib[m]
        o = np.argsort(bc, kind="stable")
        sc, bc, dc = sc[o], bc[o], dc[o]
        cnts[c] = np.bincount(bc, minlength=GL)
        per_core.append((sc, bc, dc))

    T1 = np.maximum(1, (cnts.max(axis=0) + 127) // 128)  # tiles per block
    T1off = np.concatenate([[0], np.cumsum(T1)])
    T1tot = int(T1off[-1])
    SL1 = T1tot * 128

    # L2 subgroup of each edge slot: (src>>16)*2 + (src&1)
    # per (block, sg) tile counts, core-max
    cnts2 = np.zeros((C, GL, 4), np.int64)
    for c in range(C):
        sc, bc, _ = per_core[c]
        sg = ((sc >> 16) * 2 + (sc & 1)).astype(np.int64)
        np.add.at(cnts2[c], (bc, sg), 1)
    T2 = np.maximum(1, (cnts2.max(axis=0) + 127) // 128)  # [GL, 4] tiles
    # slot layout: sg-major, block-minor
    T2sg = T2.sum(axis=0)  # tiles per sg
    sgbase = np.concatenate([[0], np.cumsum(T2sg)]) * 128
    # within sg, block offsets
    blkoff2 = np.zeros((4, GL), np.int64)
    for s in range(4):
        blkoff2[s] = sgbase[s] + np.concatenate([[0], np.cumsum(T2[:, s])])[:-1] * 128
    SL2 = int(sgbase[-1])

    in_maps = []
    for c in range(C):
        sc, bc, dc = per_core[c]
        # ---- L1 slot assignment (block-major, pad each block to T1[b]*128)
        slot_src = np.zeros(SL1, np.int64)
        slot_d = np.full(SL1, -1.0, np.float32)
        slot_s = np.zeros(SL1, np.float32)
        boff = np.concatenate([[0], np.cumsum(cnts[c])])[:-1]
        pos = (T1off[bc] * 128) + (np.arange(sc.shape[0]) - boff[bc])
        slot_src[pos] = sc
        slot_d[pos] = dc
        slot_s[pos] = dis_h[sc]

        xe1 = x_bf[slot_src]  # [SL1, 128]
        xe1 = np.ascontiguousarray(
            xe1.reshape(T1tot, 128, FEAT).transpose(1, 0, 2).reshape(128, T1tot * FEAT)
        )
        oh1 = np.zeros((SL1, 128), BFNP)
        v1 = slot_d >= 0
        oh1[np.nonzero(v1)[0], slot_d[v1].astype(np.int64)] = slot_s[v1].astype(BFNP)
        oh1 = np.ascontiguousarray(
            oh1.reshape(T1tot, 128, 128).transpose(1, 0, 2).reshape(128, T1tot * 128)
        )

        # ---- L2 slot assignment (sg-major, block-minor, pad per (b, sg))
        sg = ((sc >> 16) * 2 + (sc & 1)).astype(np.int64)
        slot2_m = np.zeros(SL2, np.int64)
        slot2_d = np.full(SL2, -1.0, np.float32)
        # rank within (b, sg)
        key = bc * 4 + sg
        o2 = np.argsort(key, kind="stable")
        ks = key[o2]
        rank = np.arange(ks.shape[0]) - np.searchsorted(ks, ks)
        pos2 = np.empty_like(rank)
        pos2[o2] = blkoff2[sg[o2], bc[o2]] + rank
        slot2_m[pos2] = (sc & 65535) >> 1
        slot2_d[pos2] = dc
        gidx2 = np.concatenate(
            [
                _wrap16(slot2_m[sgbase[s] : sgbase[s + 1]].astype(np.int16)[None, :])
                for s in range(4)
            ],
            axis=1,
        )
        T2tot_ = SL2 // 128
        oh2 = np.zeros((SL2, 128), BFNP)
        v2 = slot2_d >= 0
        oh2[np.nonzero(v2)[0], slot2_d[v2].astype(np.int64)] = BFNP(1.0)
        oh2 = np.ascontiguousarray(
            oh2.reshape(T2tot_, 128, 128).transpose(1, 0, 2).reshape(128, T2tot_ * 128)
        )

        in_maps.append(
            dict(
                xe1=xe1,
                oh1=oh1,
                gidx2=np.ascontiguousarray(gidx2),
                oh2=oh2,
                degp_l=degp_l[c],
                w1=np.ascontiguousarray(W1.astype(BFNP)),
                w2=np.ascontiguousarray(W2),
                b1r=np.ascontiguousarray(np.tile(b1[None, :], (128, 1))),
                b2r=np.ascontiguousarray(np.tile(b2[None, :], (128, 1))),
            )
        )

    # ---- final pairs (baseline machinery)
    pq = np.concatenate([pe, ne], axis=1)
    P = pq.shape[1]
    PC = P // C
    a = pq[0].reshape(C, PC)
    b = pq[1].reshape(C, PC)
    fkey = (a & 3) * 4 + (b & 3)
    forder = np.argsort(fkey, axis=1, kind="stable")
    fks = np.take_along_axis(fkey, forder, axis=1)
    a_s = np.take_along_axis(a, forder, axis=1)
    b_s = np.take_along_axis(b, forder, axis=1)
    fbounds = np.stack([np.searchsorted(fks[c], np.arange(17)) for c in range(C)])
    fcounts = fbounds[:, 1:] - fbounds[:, :-1]
    TILE_F = int(math.ceil(fcounts.max() / 128)) * 128  # one instr per sg
    n_ft = 1
    F_sub = n_ft * TILE_F

    fA = np.empty((C, 16, F_sub), np.int16)
    fB = np.empty((C, 16, F_sub), np.int16)
    TJ = TILE_F // 128
    i = np.arange(F_sub)
    t_i = i // TILE_F
    r = i % TILE_F
    lin_i = t_i * TILE_F + (r % 128) * TJ + (r // 128)
    out_pos = np.empty((C, 16 * F_sub), np.int64)
    out_src = np.empty((C, 16 * F_sub), np.int64)
    for c in range(C):
        for s in range(16):
            b0, b1_ = fbounds[c, s], fbounds[c, s + 1]
            cnt = b1_ - b0
            pad = np.arange(F_sub - cnt, dtype=np.int64) % 128
            fA[c, s, :cnt] = a_s[c, b0:b1_] >> 2
            fA[c, s, cnt:] = pad
            fB[c, s, :cnt] = b_s[c, b0:b1_] >> 2
            fB[c, s, cnt:] = pad
            base = s * F_sub
            out_pos[c, base : base + F_sub] = s * n_ft * TILE_F + lin_i
            osrc = np.full(F_sub, -1, np.int64)
            osrc[:cnt] = c * PC + forder[c, b0:b1_]
            out_src[c, base : base + F_sub] = osrc
    fidxA = _wrap16(fA)
    fidxB = _wrap16(fB)
    for c in range(C):
        in_maps[c]["fidxA"] = np.ascontiguousarray(fidxA[c])
        in_maps[c]["fidxB"] = np.ascontiguousarray(fidxB[c])

    meta = dict(
        T1=tuple(int(t) for t in T1),
        T2=tuple(tuple(int(t) for t in row) for row in T2),
        n_ft=n_ft,
        TILE_F=TILE_F,
        P=P,
        out_pos=out_pos,
        out_src=out_src,
    )
    return in_maps, meta


def assemble(out_maps, meta, cfg):
    P = meta["P"]
    logits = np.zeros(P, np.float32)
    for c in range(cfg["C"]):
        lraw = out_maps[c]["lraw"].reshape(-1)
        pos = meta["out_pos"][c]
        srcg = meta["out_src"][c]
        valid = srcg >= 0
        logits[srcg[valid]] = lraw[pos[valid]]
    return logits


# ---------------------------------------------------------------- device build


def build(cfg, meta, enable_asserts=False):
    d = cfg
    C = d["C"]
    FEAT, HID, OUT = d["FEAT"], d["HID"], d["OUT"]
    S, NP, GL = d["S"], d["NP"], d["GL"]
    TILE_F = meta["TILE_F"]
    T1 = meta["T1"]
    T2 = meta["T2"]
    n_ft = meta["n_ft"]
    F_sub = n_ft * TILE_F
    TJ_F = TILE_F // 128
    T1tot = sum(T1)
    T2sg = [sum(T2[b][s] for b in range(GL)) for s in range(4)]
    T2tot = sum(T2sg)
    GCAP = d["GCAP"]

    nc = bacc.Bacc(
        "TRN2",
        target_bir_lowering=False,
        debug=False,
        enable_asserts=enable_asserts,
        num_devices=C,
        dynamic_dma_scratch_size=d["DMA_SCRATCH"],
        num_swdge_queues=4,
    )

    # I/O
    xe1 = nc.dram_tensor("xe1", [128, T1tot * FEAT], BF16, kind="ExternalInput")
    oh1 = nc.dram_tensor("oh1", [128, T1tot * 128], BF16, kind="ExternalInput")
    gidx2 = nc.dram_tensor("gidx2", [128, T2tot * 8], I16, kind="ExternalInput")
    oh2 = nc.dram_tensor("oh2", [128, T2tot * 128], BF16, kind="ExternalInput")
    degp_l = nc.dram_tensor("degp_l", [128, GL], F32, kind="ExternalInput")
    w1 = nc.dram_tensor("w1", [FEAT, HID], BF16, kind="ExternalInput")
    w2 = nc.dram_tensor("w2", [HID, OUT], F32, kind="ExternalInput")
    b1r = nc.dram_tensor("b1r", [128, HID], F32, kind="ExternalInput")
    b2r = nc.dram_tensor("b2r", [128, OUT], F32, kind="ExternalInput")
    fidxA = nc.dram_tensor("fidxA", [128, F_sub], I16, kind="ExternalInput")
    fidxB = nc.dram_tensor("fidxB", [128, F_sub], I16, kind="ExternalInput")
    lraw = nc.dram_tensor("lraw", [16 * F_sub], F32, kind="ExternalOutput")

    # internal DRAM
    zn1_sh = nc.dram_tensor("zn1_sh", [S * HID], BF16)
    zn1_t = nc.dram_tensor("zn1_t", [d["ZTAB"]], BF16, addr_space="Shared")
    z2_sh = nc.dram_tensor("z2_sh", [S * OUT], F32)
    z2_t = nc.dram_tensor("z2_t", [d["NTAB2F"]], F32, addr_space="Shared")

    groups = [list(range(C))]

    def zn1_view(sub):
        c, p = sub >> 1, sub & 1
        base = c * 65536 * HID + p * HID
        return zn1_t.ap()[base : base + 32768 * 128].rearrange("(m e) -> m e", e=128)

    def tab2_view(t, par):
        return t.ap()[par * OUT : par * OUT + d["M2"] * HID].rearrange(
            "(m e) -> m e", e=HID
        )

    with tile.TileContext(nc) as tc:
        with (
            tc.tile_pool(name="persist", bufs=1) as pP,
            tc.tile_pool(name="idx", bufs=4) as pIdx,
        ):
            # ---- persistent small tensors
            w1_sb = pP.tile([FEAT, HID], BF16)
            nc.sync.dma_start(out=w1_sb[:], in_=w1[:, :])
            w2_sb = pP.tile([HID, OUT], F32)
            nc.sync.dma_start(out=w2_sb[:], in_=w2[:, :])
            b1_sb = pP.tile([128, HID], F32)
            nc.sync.dma_start(out=b1_sb[:], in_=b1r[:, :])
            b2_sb = pP.tile([128, OUT], F32)
            nc.sync.dma_start(out=b2_sb[:], in_=b2r[:, :])
            ident = pP.tile([128, 128], F32)
            make_identity(nc, ident[:])

            dl_raw = pP.tile([128, GL], F32)
            nc.sync.dma_start(out=dl_raw[:], in_=degp_l[:, :])
            dis_l = pP.tile([128, GL], F32)
            nc.vector.reciprocal(dis_l[:], dl_raw[:])
            nc.scalar.activation(dis_l[:], dis_l[:], AF.Sqrt)


            # mid-lived tensors: freed before the final phase to fit SBUF
            with tc.tile_pool(name="mid", bufs=1) as pM:
                zn1_local = pM.tile([128, GL * HID], BF16)
                z2_local = pM.tile([128, GL * OUT], F32)
                t1T_sb = pM.tile([HID, S], F32)
                nc.vector.memset(t1T_sb[:], 0.0)

                # ---- zero z2 table tail (strided pair views read past NP*OUT)
                ZCOLS = 4096
                with tc.tile_pool(name="zero", bufs=1) as pZ:
                    zsb = pZ.tile([128, ZCOLS], F32)
                    nc.vector.memset(zsb[:], 0.0)
                    flat = z2_t.ap()
                    off = NP * OUT
                    n_floats = d["NTAB2F"] - off
                    assert n_floats % 128 == 0
                    while n_floats > 0:
                        f = min(ZCOLS, n_floats // 128)
                        nc.sync.dma_start(
                            out=flat[off : off + 128 * f].rearrange("(p f) -> p f", f=f),
                            in_=zsb[:, 0:f],
                        )
                        off += 128 * f
                        n_floats -= 128 * f

                # ---- L1: stream x_edge, one-hot aggregate, per-block epilogue
                with (
                    tc.tile_pool(name="l1s", bufs=3) as pS,
                    tc.tile_pool(name="l1oh", bufs=3) as pOh,
                    tc.tile_pool(name="l1e", bufs=3) as pC1,
                    tc.tile_pool(name="psA", bufs=2, space="PSUM") as psA,
                    tc.tile_pool(name="psE", bufs=2, space="PSUM") as psE,
                ):
                    coff = 0
                    for b in range(GL):
                        Tb = T1[b]
                        xe_sb = pS.tile([128, Tb * FEAT], BF16, tag="xe")
                        nc.sync.dma_start(
                            out=xe_sb[:], in_=xe1[:, coff * FEAT : (coff + Tb) * FEAT]
                        )
                        oh = pOh.tile([128, Tb * 128], BF16, tag="oh")
                        nc.sync.dma_start(
                            out=oh[:], in_=oh1[:, coff * 128 : (coff + Tb) * 128]
                        )
                        ps = psA.tile([128, 128], F32, tag="agg")
                        for t in range(Tb):
                            nc.tensor.matmul(
                                ps[:],
                                lhsT=xe_sb[:, t * FEAT : (t + 1) * FEAT],
                                rhs=oh[:, t * 128 : (t + 1) * 128],
                                start=(t == 0),
                                stop=(t == Tb - 1),
                            )
                        # epilogue: aggT [f, d] -> z1preT = W1^T@aggT -> transpose
                        aggT_sb = pC1.tile([128, 128], BF16, tag="aggT")
                        nc.vector.tensor_copy(aggT_sb[:], ps[:])
                        ps_z = psE.tile([HID, 128], F32, tag="psz")
                        nc.tensor.matmul(
                            ps_z[:], lhsT=w1_sb[:], rhs=aggT_sb[:], start=True, stop=True
                        )
                        zpT_sb = pC1.tile([HID, 128], F32, tag="zpT")
                        nc.vector.tensor_copy(zpT_sb[:], ps_z[:])
                        ps_t = psE.tile([128, HID], F32, tag="pst")
                        nc.tensor.transpose(ps_t[:], zpT_sb[:], ident[0:HID, 0:HID])
                        z1 = pC1.tile([128, HID], F32, tag="z1")
                        nc.vector.tensor_scalar(
                            out=z1[:],
                            in0=ps_t[:],
                            scalar1=dis_l[:, b : b + 1],
                            scalar2=None,
                            op0=ALU.mult,
                        )
                        nc.vector.tensor_tensor(out=z1[:], in0=z1[:], in1=b1_sb[:], op=ALU.add)
                        nc.scalar.activation(z1[:], z1[:], AF.Relu)
                        nc.vector.tensor_scalar(
                            out=zn1_local[:, b * HID : (b + 1) * HID],
                            in0=z1[:],
                            scalar1=dis_l[:, b : b + 1],
                            scalar2=None,
                            op0=ALU.mult,
                        )
                        coff += Tb

                nc.sync.dma_start(
                    out=zn1_sh.ap().rearrange("(g p f) -> p g f", p=128, f=HID),
                    in_=zn1_local[:].rearrange("p (g f) -> p g f", f=HID),
                )
                nc.gpsimd.collective_compute(
                    "AllGather",
                    ALU.bypass,
                    replica_groups=groups,
                    ins=[zn1_sh.ap()],
                    outs=[zn1_t.ap()[0 : NP * HID]],
                )

                # ---- L2: gather zn1 rows (block-grouped), one-hot aggregate
                with (
                    tc.tile_pool(name="msg", bufs=4) as pMsg,
                    tc.tile_pool(name="l2oh", bufs=4) as pOh2,
                    tc.tile_pool(name="psB", bufs=4, space="PSUM") as psB,
                ):
                    md2off = 0  # in tiles
                    qi = 0
                    for s in range(4):
                        # batch whole blocks into gather instructions <= GCAP slots
                        runs = []
                        run = []
                        slots = 0
                        for b in range(GL):
                            tb = T2[b][s]
                            if slots + tb * 128 > GCAP and run:
                                runs.append(run)
                                run, slots = [], 0
                            run.append(b)
                            slots += tb * 128
                        if run:
                            runs.append(run)
                        goff = sum(T2sg[ss] for ss in range(s)) * 8  # idx cols so far
                        for run in runs:
                            rslots = sum(T2[b][s] for b in run) * 128
                            gi = pIdx.tile([128, rslots // 16], I16, tag="gi")
                            nc.sync.dma_start(
                                out=gi[:], in_=gidx2[:, goff : goff + rslots // 16]
                            )
                            goff += rslots // 16
                            msg = pMsg.tile([128, rslots // 128, 128], BF16, tag="msg")
                            nc.gpsimd.dma_gather(
                                msg[:], zn1_view(s), gi[:], rslots, rslots, 128,
                                single_packet=rslots <= 1024,
                                queue_num=qi,
                            )
                            qi = (qi + 1) % 4
                            rtiles = rslots // 128
                            oh2_sb = pOh2.tile([128, rtiles * 128], BF16, tag="oh2")
                            nc.sync.dma_start(
                                out=oh2_sb[:],
                                in_=oh2[:, md2off * 128 : (md2off + rtiles) * 128],
                            )
                            j = 0
                            for b in run:
                                tb = T2[b][s]
                                ps2 = psB.tile([HID, 128], F32, tag="t1z")
                                for t in range(tb):
                                    nc.tensor.matmul(
                                        ps2[:],
                                        lhsT=msg[:, j + t, 0:HID],
                                        rhs=oh2_sb[:, (j + t) * 128 : (j + t + 1) * 128],
                                        start=(t == 0),
                                        stop=(t == tb - 1),
                                    )
                                nc.vector.tensor_tensor(
                                    out=t1T_sb[:, b * 128 : (b + 1) * 128],
                                    in0=t1T_sb[:, b * 128 : (b + 1) * 128],
                                    in1=ps2[:],
                                    op=ALU.add,
                                )
                                j += tb
                            md2off += j

                # ---- L2 epilogue per block: z2 = dis*(t1z @ W2) + b2
                with (
                    tc.tile_pool(name="l2e", bufs=3) as pC2,
                    tc.tile_pool(name="psF", bufs=2, space="PSUM") as psF,
                ):
                    for b in range(GL):
                        ps_q = psF.tile([OUT, 128], F32, tag="psq")
                        nc.tensor.matmul(
                            ps_q[:],
                            lhsT=w2_sb[:],
                            rhs=t1T_sb[:, b * 128 : (b + 1) * 128],
                            start=True,
                            stop=True,
                        )
                        q_sb = pC2.tile([OUT, 128], F32, tag="qsb")
                        nc.vector.tensor_copy(q_sb[:], ps_q[:])
                        ps_q2 = psF.tile([128, OUT], F32, tag="psq2")
                        nc.tensor.transpose(ps_q2[:], q_sb[:], ident[0:OUT, 0:OUT])
                        nc.vector.tensor_scalar(
                            out=z2_local[:, b * OUT : (b + 1) * OUT],
                            in0=ps_q2[:],
                            scalar1=dis_l[:, b : b + 1],
                            scalar2=None,
                            op0=ALU.mult,
                        )
                        nc.vector.tensor_tensor(
                            out=z2_local[:, b * OUT : (b + 1) * OUT],
                            in0=z2_local[:, b * OUT : (b + 1) * OUT],
                            in1=b2_sb[:],
                            op=ALU.add,
                        )

                nc.sync.dma_start(
                    out=z2_sh.ap().rearrange("(g p f) -> p g f", p=128, f=OUT),
                    in_=z2_local[:].rearrange("p (g f) -> p g f", f=OUT),
                )
                nc.gpsimd.collective_compute(
                    "AllGather",
                    ALU.bypass,
                    replica_groups=groups,
                    ins=[z2_sh.ap()],
                    outs=[z2_t.ap()[0 : NP * OUT]],
                )


            # ---- final: edge logits (baseline machinery)
            with tc.tile_pool(name="fin", bufs=3) as pFin:
                colsF = TILE_F // 16
                for s in range(16):
                    for t in range(n_ft):
                        off16 = (s * n_ft + t) * colsF
                        fa = pIdx.tile([128, colsF], I16, tag="fa")
                        nc.sync.dma_start(
                            out=fa[:], in_=fidxA[:, off16 : off16 + colsF]
                        )
                        fb = pIdx.tile([128, colsF], I16, tag="fb")
                        nc.sync.dma_start(
                            out=fb[:], in_=fidxB[:, off16 : off16 + colsF]
                        )
                        ma = pFin.tile([128, TJ_F, HID], F32, tag="ma")
                        nc.gpsimd.dma_gather(
                            ma[:], tab2_view(z2_t, s >> 2), fa[:], TILE_F, TILE_F, HID,
                            single_packet=TILE_F <= 1024,
                            queue_num=(2 * s) % 4,
                        )
                        mb = pFin.tile([128, TJ_F, HID], F32, tag="mb")
                        nc.gpsimd.dma_gather(
                            mb[:], tab2_view(z2_t, s & 3), fb[:], TILE_F, TILE_F, HID,
                            single_packet=TILE_F <= 1024,
                            queue_num=(2 * s + 1) % 4,
                        )
                        prod = pFin.tile([128, TJ_F, OUT], F32, tag="prod")
                        nc.vector.tensor_tensor(
                            out=prod[:],
                            in0=ma[:, :, 0:OUT],
                            in1=mb[:, :, 0:OUT],
                            op=ALU.mult,
                        )
                        red = pFin.tile([128, TJ_F], F32, tag="red")
                        nc.vector.reduce_sum(
                            out=red[:, :, None],
                            in_=prod[:],
                            axis=mybir.AxisListType.X,
                        )
                        blk = s * n_ft + t
                        nc.sync.dma_start(
                            out=lraw.ap()[
                                blk * TILE_F : (blk + 1) * TILE_F
                            ].rearrange("(p j) -> p j", j=TJ_F),
                            in_=red[:],
                        )

    nc.compile()
    return nc


# ---------------------------------------------------------------- entry point

_CACHE = {}
TRACE = False
LAST = {}


def kernel(**inputs):
    cfg = derive(default_cfg())
    in_maps, meta = prep_host(inputs, cfg)
    key = (meta["T1"], meta["T2"], meta["n_ft"], meta["TILE_F"])
    if key not in _CACHE:
        _CACHE[key] = build(cfg, meta)
    nc = _CACHE[key]
    res = bass_utils.run_bass_kernel_spmd(
        nc, in_maps, core_ids=list(range(cfg["C"])), trace=TRACE
    )
    LAST["res"] = res
    return assemble(res.results, meta, cfg)

